# revision 13
# baseline (speedup 1.0000x reference)
"""GAT-style edge-softmax self-attention, dense-mask formulation, 8-core SPMD.

Math: per batch b (one NeuronCore per batch),
  Q/K/V = X @ Wq/k/v ; per head h: S = Q_h K_h^T / 8
  ex = C * exp(S)           (C[i,j] = multiplicity of edge (i<-j); softmax is
                             shift-invariant and |S| <~ 7, so no row-max needed)
  out_i = (ex @ V)_i / max(sum_j ex_ij, 1e-9)

v4 design notes (all per core):
  - scores: head PAIRS via PE row tiling (K=64, tile_position (0,0)/(64,0))
    into a 2-bank PSUM group, double-buffered so scores of chunk j+1 overlap
    exp/mult/AV of chunk j.
  - exp: one ACT instruction per 2-bank group (free dim 1024).
  - mask multiply: all-bf16 tensor_tensor (DVE 2x), mask block broadcast
    across the 2 heads via a stride-0 AP dim.
  - AV: V stationary (64 cols + ones column -> softmax denominator), exm
    streams n=512.  Output [feature, node] in PSUM.
  - output: DVE copy to bf16 SBUF, DMA-transpose (xbar) back to [node,
    feature], reciprocal-normalize, assemble in a bf16 SBUF buffer; host
    upcasts to float32.
  - Q/K projection chunks are emitted INSIDE the first attention pass so
    the PE's in-order stream interleaves them with attention matmuls (the
    attention is ACT/DVE-paced; projections fill the PE bubbles).
"""

import numpy as np
import ml_dtypes

import concourse.bass as bass
import concourse.bacc as bacc
import concourse.mybir as mybir
import concourse.tile as tile
from concourse.bass_utils import run_bass_kernel_spmd

B, N, H = 8, 1024, 768
NH, HD = 12, 64
P = 128
KC = H // P   # 6 contraction chunks for projections
JC = N // P   # 8 node chunks
NHP = NH // 2  # head pairs
VW = 80        # per-head stride in vp / AV output rows (16-aligned)
F32 = mybir.dt.float32
BF16 = mybir.dt.bfloat16

_CACHE = {}


def _build_nc():
    nc = bacc.Bacc("TRN2", target_bir_lowering=False, debug=True)

    xT_d = nc.dram_tensor("xT", [H, N], BF16, kind="ExternalInput")
    wq_d = nc.dram_tensor("wq", [H, H], BF16, kind="ExternalInput")
    wk_d = nc.dram_tensor("wk", [H, H], BF16, kind="ExternalInput")
    wv_d = nc.dram_tensor("wv", [H, H], BF16, kind="ExternalInput")
    # mask, device layout: [p, i2*4096 + jc*512 + io] (j = jc*128+p, i = i2*512+io)
    mT_d = nc.dram_tensor("maskT", [P, JC * N], BF16, kind="ExternalInput")
    out_d = nc.dram_tensor("out", [N, H], BF16, kind="ExternalOutput")

    with tile.TileContext(nc) as tc:
        with tc.tile_pool(name="res", bufs=1) as res, \
             tc.tile_pool(name="work", bufs=3) as work, \
             tc.tile_pool(name="pps", bufs=2, space="PSUM") as pps, \
             tc.tile_pool(name="sps", bufs=2, space="PSUM") as spsp, \
             tc.tile_pool(name="ops", bufs=1, space="PSUM") as opsp:

            # ---- resident loads ----
            xT = [res.tile([P, N], BF16, tag=f"xT{k}", name=f"xT{k}") for k in range(KC)]
            wq = [res.tile([P, H], BF16, tag=f"wq{k}", name=f"wq{k}") for k in range(KC)]
            wk = [res.tile([P, H], BF16, tag=f"wk{k}", name=f"wk{k}") for k in range(KC)]
            wv = [res.tile([P, H], BF16, tag=f"wv{k}", name=f"wv{k}") for k in range(KC)]
            mT = res.tile([P, JC * N], BF16, tag="mT", name="mT")
            for k in range(KC):
                nc.default_dma_engine.dma_start(out=xT[k][:], in_=xT_d[k * P:(k + 1) * P, :])
                nc.default_dma_engine.dma_start(out=wq[k][:], in_=wq_d[k * P:(k + 1) * P, :])
                nc.default_dma_engine.dma_start(out=wk[k][:], in_=wk_d[k * P:(k + 1) * P, :])
                nc.default_dma_engine.dma_start(out=wv[k][:], in_=wv_d[k * P:(k + 1) * P, :])
            for j in range(JC):
                nc.default_dma_engine.dma_start(
                    out=mT[:, j * N:(j + 1) * N], in_=mT_d[:, j * N:(j + 1) * N])

            # computed residents
            qT = [res.tile([P, N], BF16, tag=f"qT{k}", name=f"qT{k}") for k in range(KC)]
            kT = [res.tile([P, N], BF16, tag=f"kT{k}", name=f"kT{k}") for k in range(KC)]
            # V packed per head, stride 80: cols 0-63 V, col 64 ones
            # (denominator trick), cols 65-79 zero pad (keeps the AV output
            # 16-row aligned for the DMA-transpose xbar)
            vp = [res.tile([P, NH * VW], BF16, tag=f"vp{j}", name=f"vp{j}") for j in range(JC)]
            # final output, [p, ic*768 + h*64 + f], bf16
            outt = res.tile([P, JC * H], BF16, tag="outt", name="outt")

            # ---- projections: V first ----
            for j in range(JC):
                nc.gpsimd.memset(vp[j][:], 0.0)
                nc.gpsimd.memset(
                    vp[j][:].rearrange("p (h x) -> p h x", h=NH)[:, :, HD:HD + 1], 1.0)
                for nn, (c0, cw, nh) in enumerate(((0, 512, 8), (512, 256, 4))):
                    ps = pps.tile([P, 512], F32, tag="proj")
                    for k in range(KC):
                        nc.tensor.matmul(
                            ps[:, :cw],
                            xT[k][:, j * P:(j + 1) * P],
                            wv[k][:, c0:c0 + cw],
                            start=(k == 0), stop=(k == KC - 1),
                        )
                    h0 = c0 // HD
                    src = ps[:, 0:cw].rearrange("p (h x) -> p h x", h=nh)
                    dst = vp[j][:, h0 * VW:(h0 + nh) * VW] \
                        .rearrange("p (h x) -> p h x", h=nh)[:, :, 0:HD]
                    nc.scalar.activation(
                        dst, src, mybir.ActivationFunctionType.Copy)

            def qk_chunk(mo):
                for w_sb, dst in ((wq, qT), (wk, kT)):
                    for nn in range(2):
                        ps = pps.tile([P, 512], F32, tag="proj")
                        for k in range(KC):
                            nc.tensor.matmul(
                                ps[:],
                                w_sb[k][:, mo * P:(mo + 1) * P],
                                xT[k][:, nn * 512:(nn + 1) * 512],
                                start=(k == 0), stop=(k == KC - 1),
                            )
                        nc.scalar.activation(
                            dst[mo][:, nn * 512:(nn + 1) * 512], ps[:],
                            mybir.ActivationFunctionType.Copy)

            qk_chunk(0)

            # ---- main attention loop ----
            for i2 in range(2):
                for hp in range(NHP):
                    if i2 == 0 and hp + 1 < KC:
                        qk_chunk(hp + 1)  # interleave remaining projections
                    hA, hB = 2 * hp, 2 * hp + 1
                    kt, qt = kT[hp], qT[hp]
                    oAB = [opsp.tile([P, 512], F32, tag=f"o{x}", name=f"o{x}_{hp}_{i2}")
                           for x in "AB"]
                    for j in range(JC):
                        S2 = spsp.tile([P, 1024], F32, tag="S2")
                        nc.tensor.matmul(
                            S2[:, 0:512],
                            kt[0:HD, j * P:(j + 1) * P],
                            qt[0:HD, i2 * 512:(i2 + 1) * 512],
                            start=True, stop=True, tile_position=(0, 0))
                        nc.tensor.matmul(
                            S2[:, 512:1024],
                            kt[HD:P, j * P:(j + 1) * P],
                            qt[HD:P, i2 * 512:(i2 + 1) * 512],
                            start=True, stop=True, tile_position=(64, 0))
                        EX = work.tile([P, 1024], BF16, tag="EX")
                        nc.scalar.activation(
                            EX[:], S2[:],
                            mybir.ActivationFunctionType.Exp, scale=0.125)
                        XM = work.tile([P, 1024], BF16, tag="XM")
                        base = i2 * 4096 + j * 512
                        m_ap = mT[:, base:base + 512] \
                            .unsqueeze(1).broadcast_to((P, 2, 512))
                        nc.vector.tensor_tensor(
                            out=XM[:].rearrange("p (h x) -> p h x", h=2),
                            in0=EX[:].rearrange("p (h x) -> p h x", h=2),
                            in1=m_ap, op=mybir.AluOpType.mult)
                        first, last = (j == 0), (j == JC - 1)
                        nc.tensor.matmul(
                            oAB[0][0:VW, :],
                            vp[j][:, hA * VW:(hA + 1) * VW],
                            XM[:, 0:512], start=first, stop=last)
                        nc.tensor.matmul(
                            oAB[1][0:VW, :],
                            vp[j][:, hB * VW:(hB + 1) * VW],
                            XM[:, 512:1024], start=first, stop=last)
                    # output: DMA-transpose + normalize per head
                    for h, o in ((hA, oAB[0]), (hB, oAB[1])):
                        oraw = work.tile([P, 512], BF16, tag="oraw")
                        nc.vector.tensor_copy(out=oraw[0:VW, :], in_=o[0:VW, :])
                        oT = work.tile([P, 4 * VW], BF16, tag="oT")
                        for s in range(4):
                            nc.sync.dma_start_transpose(
                                out=oT[:, s * VW:(s + 1) * VW],
                                in_=oraw[0:VW, s * P:(s + 1) * P])
                        rec = work.tile([P, 4], BF16, tag="rec")
                        den_ap = oT[:].rearrange("p (s x) -> p s x", s=4)[:, :, HD:HD + 1]
                        with nc.allow_low_precision(reason="bf16 softmax denom is ample"):
                            nc.vector.tensor_scalar_max(rec[:].unsqueeze(2), den_ap, 1e-9)
                            nc.vector.reciprocal(rec[:], rec[:])
                        src = oT[:].rearrange("p (s x) -> p s x", s=4)[:, :, 0:HD]
                        r_b = rec[:].unsqueeze(2).broadcast_to((P, 4, HD))
                        dst = outt[:, i2 * 4 * H:(i2 + 1) * 4 * H] \
                            .rearrange("p (s x) -> p s x", s=4)[:, :, h * HD:(h + 1) * HD]
                        nc.vector.tensor_tensor(
                            out=dst, in0=src, in1=r_b, op=mybir.AluOpType.mult)
                # this i-half is complete for all heads: stream it out
                for s in range(4):
                    ic = i2 * 4 + s
                    nc.default_dma_engine.dma_start(
                        out=out_d[ic * P:(ic + 1) * P, :],
                        in_=outt[:, ic * H:(ic + 1) * H])

    nc.compile()
    return nc


def _prep_in_maps(node_states, edge_indices, Wq, Wk, Wv):
    eb, ei, ej = (np.asarray(edge_indices[r]) for r in range(3))
    idx = (eb.astype(np.int64) * N + ej) * N + ei
    CT = np.bincount(idx, minlength=B * N * N).astype(np.float32).reshape(B, N, N)
    # device mask layout: [p, i2*4096 + jc*512 + io]
    CTd = CT.reshape(B, JC, P, 2, 512).transpose(0, 2, 3, 1, 4).reshape(B, P, JC * N)

    bf = ml_dtypes.bfloat16
    wq = np.ascontiguousarray(Wq).astype(bf)
    wk = np.ascontiguousarray(Wk).astype(bf)
    wv = np.ascontiguousarray(Wv).astype(bf)

    in_maps = []
    for b in range(B):
        in_maps.append({
            "xT": np.ascontiguousarray(np.asarray(node_states[b]).T).astype(bf),
            "wq": wq, "wk": wk, "wv": wv,
            "maskT": np.ascontiguousarray(CTd[b]).astype(bf),
        })
    return in_maps


def kernel(node_states, edge_indices, Wq, Wk, Wv):
    if "nc" not in _CACHE:
        _CACHE["nc"] = _build_nc()
    nc = _CACHE["nc"]
    in_maps = _prep_in_maps(node_states, edge_indices, Wq, Wk, Wv)
    res = run_bass_kernel_spmd(nc, in_maps, list(range(B)))
    out = np.stack([np.asarray(res.results[b]["out"]) for b in range(B)], axis=0)
    return out.astype(np.float32)


def run_traced(inputs, **kw):
    if "nc" not in _CACHE:
        _CACHE["nc"] = _build_nc()
    nc = _CACHE["nc"]
    in_maps = _prep_in_maps(**inputs)
    return run_bass_kernel_spmd(nc, in_maps, list(range(B)), trace=True, **kw)


# revision 14
# speedup vs baseline: 1.2459x; 1.2459x over previous
"""GAT-style edge-softmax self-attention, dense-mask formulation, 8-core SPMD.

Math: per batch b (one NeuronCore per batch),
  Q/K/V = X @ Wq/k/v ; per head h: S = Q_h K_h^T / 8
  ex = C * exp(S)           (C[i,j] = multiplicity of edge (i<-j); softmax is
                             shift-invariant and |S| <~ 7, so no row-max needed)
  out_i = (ex @ V)_i / max(sum_j ex_ij, 1e-9)

v4 design notes (all per core):
  - scores: head PAIRS via PE row tiling (K=64, tile_position (0,0)/(64,0))
    into a 2-bank PSUM group, double-buffered so scores of chunk j+1 overlap
    exp/mult/AV of chunk j.
  - exp: one ACT instruction per 2-bank group (free dim 1024).
  - mask multiply: all-bf16 tensor_tensor (DVE 2x), mask block broadcast
    across the 2 heads via a stride-0 AP dim.
  - AV: V stationary (64 cols + ones column -> softmax denominator), exm
    streams n=512.  Output [feature, node] in PSUM.
  - output: DVE copy to bf16 SBUF, DMA-transpose (xbar) back to [node,
    feature], reciprocal-normalize, assemble in a bf16 SBUF buffer; host
    upcasts to float32.
  - Q/K projection chunks are emitted INSIDE the first attention pass so
    the PE's in-order stream interleaves them with attention matmuls (the
    attention is ACT/DVE-paced; projections fill the PE bubbles).
"""

import numpy as np
import ml_dtypes

import concourse.bass as bass
import concourse.bacc as bacc
import concourse.mybir as mybir
import concourse.tile as tile
from concourse.bass_utils import run_bass_kernel_spmd

B, N, H = 8, 1024, 768
NH, HD = 12, 64
P = 128
KC = H // P   # 6 contraction chunks for projections
JC = N // P   # 8 node chunks
NHP = NH // 2  # head pairs
VW = 80        # per-head stride in vp / AV output rows (16-aligned)
F32 = mybir.dt.float32
BF16 = mybir.dt.bfloat16

_CACHE = {}


def _build_nc():
    nc = bacc.Bacc("TRN2", target_bir_lowering=False, debug=True)

    xT_d = nc.dram_tensor("xT", [H, N], BF16, kind="ExternalInput")
    wq_d = nc.dram_tensor("wq", [H, H], BF16, kind="ExternalInput")
    wk_d = nc.dram_tensor("wk", [H, H], BF16, kind="ExternalInput")
    wv_d = nc.dram_tensor("wv", [H, H], BF16, kind="ExternalInput")
    # mask, device layout: [p, i2*4096 + jc*512 + io] (j = jc*128+p, i = i2*512+io)
    mT_d = nc.dram_tensor("maskT", [P, JC * N], BF16, kind="ExternalInput")
    id_d = nc.dram_tensor("ident", [P, P], BF16, kind="ExternalInput")
    out_d = nc.dram_tensor("out", [N, H], BF16, kind="ExternalOutput")

    with tile.TileContext(nc) as tc:
        with tc.tile_pool(name="res", bufs=1) as res, \
             tc.tile_pool(name="work", bufs=3) as work, \
             tc.tile_pool(name="pps", bufs=2, space="PSUM") as pps, \
             tc.tile_pool(name="sps", bufs=2, space="PSUM") as spsp, \
             tc.tile_pool(name="ops", bufs=1, space="PSUM") as opsp:

            # ---- resident loads ----
            xT = [res.tile([P, N], BF16, tag=f"xT{k}", name=f"xT{k}") for k in range(KC)]
            wq = [res.tile([P, H], BF16, tag=f"wq{k}", name=f"wq{k}") for k in range(KC)]
            wk = [res.tile([P, H], BF16, tag=f"wk{k}", name=f"wk{k}") for k in range(KC)]
            wv = [res.tile([P, H], BF16, tag=f"wv{k}", name=f"wv{k}") for k in range(KC)]
            mT = res.tile([P, JC * N], BF16, tag="mT", name="mT")
            ident = res.tile([P, P], BF16, tag="ident", name="ident")
            nc.default_dma_engine.dma_start(out=ident[:], in_=id_d[:, :])
            for k in range(KC):
                nc.default_dma_engine.dma_start(out=xT[k][:], in_=xT_d[k * P:(k + 1) * P, :])
                nc.default_dma_engine.dma_start(out=wq[k][:], in_=wq_d[k * P:(k + 1) * P, :])
                nc.default_dma_engine.dma_start(out=wk[k][:], in_=wk_d[k * P:(k + 1) * P, :])
                nc.default_dma_engine.dma_start(out=wv[k][:], in_=wv_d[k * P:(k + 1) * P, :])
            for j in range(JC):
                nc.default_dma_engine.dma_start(
                    out=mT[:, j * N:(j + 1) * N], in_=mT_d[:, j * N:(j + 1) * N])

            # computed residents
            qT = [res.tile([P, N], BF16, tag=f"qT{k}", name=f"qT{k}") for k in range(KC)]
            kT = [res.tile([P, N], BF16, tag=f"kT{k}", name=f"kT{k}") for k in range(KC)]
            # V packed per head, stride 80: cols 0-63 V, col 64 ones
            # (denominator trick), cols 65-79 zero pad (keeps the AV output
            # 16-row aligned for the DMA-transpose xbar)
            vp = [res.tile([P, NH * VW], BF16, tag=f"vp{j}", name=f"vp{j}") for j in range(JC)]
            # final output, [p, ic*768 + h*64 + f], bf16
            outt = res.tile([P, JC * H], BF16, tag="outt", name="outt")

            # ---- projections: V first ----
            for j in range(JC):
                nc.gpsimd.memset(vp[j][:], 0.0)
                nc.gpsimd.memset(
                    vp[j][:].rearrange("p (h x) -> p h x", h=NH)[:, :, HD:HD + 1], 1.0)
                for nn, (c0, cw, nh) in enumerate(((0, 512, 8), (512, 256, 4))):
                    ps = pps.tile([P, 512], F32, tag="proj")
                    for k in range(KC):
                        nc.tensor.matmul(
                            ps[:, :cw],
                            xT[k][:, j * P:(j + 1) * P],
                            wv[k][:, c0:c0 + cw],
                            start=(k == 0), stop=(k == KC - 1),
                        )
                    h0 = c0 // HD
                    src = ps[:, 0:cw].rearrange("p (h x) -> p h x", h=nh)
                    dst = vp[j][:, h0 * VW:(h0 + nh) * VW] \
                        .rearrange("p (h x) -> p h x", h=nh)[:, :, 0:HD]
                    nc.scalar.activation(
                        dst, src, mybir.ActivationFunctionType.Copy)

            def qk_chunk(mo):
                for w_sb, dst in ((wq, qT), (wk, kT)):
                    for nn in range(2):
                        ps = pps.tile([P, 512], F32, tag="proj")
                        for k in range(KC):
                            nc.tensor.matmul(
                                ps[:],
                                w_sb[k][:, mo * P:(mo + 1) * P],
                                xT[k][:, nn * 512:(nn + 1) * 512],
                                start=(k == 0), stop=(k == KC - 1),
                            )
                        nc.scalar.activation(
                            dst[mo][:, nn * 512:(nn + 1) * 512], ps[:],
                            mybir.ActivationFunctionType.Copy)

            qk_chunk(0)

            # ---- main attention loop ----
            for i2 in range(2):
                for hp in range(NHP):
                    if i2 == 0 and hp + 1 < KC:
                        qk_chunk(hp + 1)  # interleave remaining projections
                    hA, hB = 2 * hp, 2 * hp + 1
                    kt, qt = kT[hp], qT[hp]
                    oAB = [opsp.tile([P, 512], F32, tag=f"o{x}", name=f"o{x}_{hp}_{i2}")
                           for x in "AB"]
                    for j in range(JC):
                        S2 = spsp.tile([P, 1024], F32, tag="S2")
                        nc.tensor.matmul(
                            S2[:, 0:512],
                            kt[0:HD, j * P:(j + 1) * P],
                            qt[0:HD, i2 * 512:(i2 + 1) * 512],
                            start=True, stop=True, tile_position=(0, 0))
                        nc.tensor.matmul(
                            S2[:, 512:1024],
                            kt[HD:P, j * P:(j + 1) * P],
                            qt[HD:P, i2 * 512:(i2 + 1) * 512],
                            start=True, stop=True, tile_position=(64, 0))
                        EX = work.tile([P, 1024], BF16, tag="EX")
                        nc.scalar.activation(
                            EX[:], S2[:],
                            mybir.ActivationFunctionType.Exp, scale=0.125)
                        XM = work.tile([P, 1024], BF16, tag="XM")
                        base = i2 * 4096 + j * 512
                        m_ap = mT[:, base:base + 512] \
                            .unsqueeze(1).broadcast_to((P, 2, 512))
                        nc.vector.tensor_tensor(
                            out=XM[:].rearrange("p (h x) -> p h x", h=2),
                            in0=EX[:].rearrange("p (h x) -> p h x", h=2),
                            in1=m_ap, op=mybir.AluOpType.mult)
                        first, last = (j == 0), (j == JC - 1)
                        nc.tensor.matmul(
                            oAB[0][0:VW, :],
                            vp[j][:, hA * VW:(hA + 1) * VW],
                            XM[:, 0:512], start=first, stop=last)
                        nc.tensor.matmul(
                            oAB[1][0:VW, :],
                            vp[j][:, hB * VW:(hB + 1) * VW],
                            XM[:, 512:1024], start=first, stop=last)
                    # output: DMA-transpose + normalize per head
                    for h, o in ((hA, oAB[0]), (hB, oAB[1])):
                        oraw = work.tile([P, 512], BF16, tag="oraw")
                        nc.vector.tensor_copy(out=oraw[0:VW, :], in_=o[0:VW, :])
                        # PE transpose back to [node, feature]; P2 shares the
                        # projection pool slots (proj is done or winding down)
                        oT = pps.tile([P, 4 * VW], BF16, tag="proj", name=f"P2_{h}_{i2}")
                        for s in range(4):
                            nc.tensor.transpose(
                                oT[:, s * VW:(s + 1) * VW],
                                oraw[0:VW, s * P:(s + 1) * P],
                                ident[0:VW, 0:VW])
                        rec = work.tile([P, 4], BF16, tag="rec")
                        den_ap = oT[:].rearrange("p (s x) -> p s x", s=4)[:, :, HD:HD + 1]
                        with nc.allow_low_precision(reason="bf16 softmax denom is ample"):
                            nc.vector.tensor_scalar_max(rec[:].unsqueeze(2), den_ap, 1e-9)
                            nc.vector.reciprocal(rec[:], rec[:])
                        src = oT[:].rearrange("p (s x) -> p s x", s=4)[:, :, 0:HD]
                        r_b = rec[:].unsqueeze(2).broadcast_to((P, 4, HD))
                        dst = outt[:, i2 * 4 * H:(i2 + 1) * 4 * H] \
                            .rearrange("p (s x) -> p s x", s=4)[:, :, h * HD:(h + 1) * HD]
                        nc.vector.tensor_tensor(
                            out=dst, in0=src, in1=r_b, op=mybir.AluOpType.mult)
                # this i-half is complete for all heads: stream it out
                for s in range(4):
                    ic = i2 * 4 + s
                    nc.default_dma_engine.dma_start(
                        out=out_d[ic * P:(ic + 1) * P, :],
                        in_=outt[:, ic * H:(ic + 1) * H])

    nc.compile()
    return nc


def _prep_in_maps(node_states, edge_indices, Wq, Wk, Wv):
    eb, ei, ej = (np.asarray(edge_indices[r]) for r in range(3))
    idx = (eb.astype(np.int64) * N + ej) * N + ei
    CT = np.bincount(idx, minlength=B * N * N).astype(np.float32).reshape(B, N, N)
    # device mask layout: [p, i2*4096 + jc*512 + io]
    CTd = CT.reshape(B, JC, P, 2, 512).transpose(0, 2, 3, 1, 4).reshape(B, P, JC * N)

    bf = ml_dtypes.bfloat16
    ident = np.eye(P, dtype=bf)
    wq = np.ascontiguousarray(Wq).astype(bf)
    wk = np.ascontiguousarray(Wk).astype(bf)
    wv = np.ascontiguousarray(Wv).astype(bf)

    in_maps = []
    for b in range(B):
        in_maps.append({
            "xT": np.ascontiguousarray(np.asarray(node_states[b]).T).astype(bf),
            "wq": wq, "wk": wk, "wv": wv,
            "maskT": np.ascontiguousarray(CTd[b]).astype(bf),
            "ident": ident,
        })
    return in_maps


def kernel(node_states, edge_indices, Wq, Wk, Wv):
    if "nc" not in _CACHE:
        _CACHE["nc"] = _build_nc()
    nc = _CACHE["nc"]
    in_maps = _prep_in_maps(node_states, edge_indices, Wq, Wk, Wv)
    res = run_bass_kernel_spmd(nc, in_maps, list(range(B)))
    out = np.stack([np.asarray(res.results[b]["out"]) for b in range(B)], axis=0)
    return out.astype(np.float32)


def run_traced(inputs, **kw):
    if "nc" not in _CACHE:
        _CACHE["nc"] = _build_nc()
    nc = _CACHE["nc"]
    in_maps = _prep_in_maps(**inputs)
    return run_bass_kernel_spmd(nc, in_maps, list(range(B)), trace=True, **kw)


# revision 15
# speedup vs baseline: 1.4410x; 1.1566x over previous
"""GAT-style edge-softmax self-attention, dense-mask formulation, 8-core SPMD.

Math: per batch b (one NeuronCore per batch),
  Q/K/V = X @ Wq/k/v ; per head h: S = Q_h K_h^T / 8
  ex = C * exp(S)           (C[i,j] = multiplicity of edge (i<-j); softmax is
                             shift-invariant and |S| <~ 7, so no row-max needed)
  out_i = (ex @ V)_i / max(sum_j ex_ij, 1e-9)

v6 design notes (per core):
  - scores: head PAIRS via PE row tiling (K=64, tile_position (0,0)/(64,0))
    into a 2-bank PSUM group, double-buffered so scores of chunk j+1 overlap
    exp/mult/AV of chunk j.
  - exp: one ACT instruction per 2-bank group (free dim 1024), reading PSUM.
  - mask multiply: all-bf16 tensor_tensor (DVE 2x), mask block broadcast
    across the 2 heads via a stride-0 AP dim.
  - AV: V stationary (64 cols + ones column -> softmax denominator), exm
    streams n=512.  Output [feature, node] in PSUM.
  - output: DVE copy to bf16 SBUF, 4x PE transpose back to [node, feature]
    (66-col blocks keep bf16 PSUM writes 4B-aligned), reciprocal-normalize
    with a stride-0 broadcast TT into a bf16 staging buffer; each i-half is
    DMA'd out as soon as all heads finished it.  Host upcasts to float32.
"""

import numpy as np
import ml_dtypes

import concourse.bass as bass
import concourse.bacc as bacc
import concourse.mybir as mybir
import concourse.tile as tile
from concourse.bass_utils import run_bass_kernel_spmd

B, N, H = 8, 1024, 768
NH, HD = 12, 64
P = 128
KC = H // P   # 6 contraction chunks for projections
JC = N // P   # 8 node chunks
NHP = NH // 2  # head pairs
VW = HD + 1    # per-head stride in vp (V cols + ones col)
SW = HD + 2    # per-s block stride in the transposed PSUM tile (4B aligned)
F32 = mybir.dt.float32
BF16 = mybir.dt.bfloat16

_CACHE = {}


def _build_nc():
    nc = bacc.Bacc("TRN2", target_bir_lowering=False, debug=True)

    xT_d = nc.dram_tensor("xT", [H, N], BF16, kind="ExternalInput")
    wq_d = nc.dram_tensor("wq", [H, H], BF16, kind="ExternalInput")
    wk_d = nc.dram_tensor("wk", [H, H], BF16, kind="ExternalInput")
    wv_d = nc.dram_tensor("wv", [H, H], BF16, kind="ExternalInput")
    # mask, device layout: [p, i2*4096 + jc*512 + io] (j = jc*128+p, i = i2*512+io)
    mT_d = nc.dram_tensor("maskT", [P, JC * N], BF16, kind="ExternalInput")
    id_d = nc.dram_tensor("ident", [P, P], BF16, kind="ExternalInput")
    out_d = nc.dram_tensor("out", [N, H], BF16, kind="ExternalOutput")

    with tile.TileContext(nc) as tc:
        with tc.tile_pool(name="res", bufs=1) as res, \
             tc.tile_pool(name="work", bufs=3) as work:

            # ---- resident loads ----
            xT = [res.tile([P, N], BF16, tag=f"xT{k}", name=f"xT{k}") for k in range(KC)]
            wq = [res.tile([P, H], BF16, tag=f"wq{k}", name=f"wq{k}") for k in range(KC)]
            wk = [res.tile([P, H], BF16, tag=f"wk{k}", name=f"wk{k}") for k in range(KC)]
            wv = [res.tile([P, H], BF16, tag=f"wv{k}", name=f"wv{k}") for k in range(KC)]
            mT = res.tile([P, JC * N], BF16, tag="mT", name="mT")
            ident = res.tile([P, P], BF16, tag="ident", name="ident")
            nc.default_dma_engine.dma_start(out=ident[:], in_=id_d[:, :])
            for k in range(KC):
                nc.default_dma_engine.dma_start(out=xT[k][:], in_=xT_d[k * P:(k + 1) * P, :])
                nc.default_dma_engine.dma_start(out=wq[k][:], in_=wq_d[k * P:(k + 1) * P, :])
                nc.default_dma_engine.dma_start(out=wk[k][:], in_=wk_d[k * P:(k + 1) * P, :])
                nc.default_dma_engine.dma_start(out=wv[k][:], in_=wv_d[k * P:(k + 1) * P, :])
            for j in range(JC):
                nc.default_dma_engine.dma_start(
                    out=mT[:, j * N:(j + 1) * N], in_=mT_d[:, j * N:(j + 1) * N])

            # computed residents
            qT = [res.tile([P, N], BF16, tag=f"qT{k}", name=f"qT{k}") for k in range(KC)]
            kT = [res.tile([P, N], BF16, tag=f"kT{k}", name=f"kT{k}") for k in range(KC)]
            vp = [res.tile([P, NH * VW], BF16, tag=f"vp{j}", name=f"vp{j}") for j in range(JC)]
            # final output, [p, ic*768 + h*64 + f], bf16
            outt = res.tile([P, JC * H], BF16, tag="outt", name="outt")

            # ---- projections ----
            with tc.tile_pool(name="pps", bufs=2, space="PSUM") as pps:
                for mo in range(KC):
                    for w_sb, dst in ((wq, qT), (wk, kT)):
                        for nn in range(2):
                            ps = pps.tile([P, 512], F32, tag="proj")
                            for k in range(KC):
                                nc.tensor.matmul(
                                    ps[:],
                                    w_sb[k][:, mo * P:(mo + 1) * P],
                                    xT[k][:, nn * 512:(nn + 1) * 512],
                                    start=(k == 0), stop=(k == KC - 1),
                                )
                            nc.scalar.activation(
                                dst[mo][:, nn * 512:(nn + 1) * 512], ps[:],
                                mybir.ActivationFunctionType.Copy)
                # V: out (j nodes, feat) = X @ Wv ; pack into vp with ones cols
                for j in range(JC):
                    nc.gpsimd.memset(vp[j][:], 1.0)
                    for nn, (c0, cw, nh) in enumerate(((0, 512, 8), (512, 256, 4))):
                        ps = pps.tile([P, 512], F32, tag="proj")
                        for k in range(KC):
                            nc.tensor.matmul(
                                ps[:, :cw],
                                xT[k][:, j * P:(j + 1) * P],
                                wv[k][:, c0:c0 + cw],
                                start=(k == 0), stop=(k == KC - 1),
                            )
                        h0 = c0 // HD
                        src = ps[:, 0:cw].rearrange("p (h x) -> p h x", h=nh)
                        dst = vp[j][:, h0 * VW:(h0 + nh) * VW] \
                            .rearrange("p (h x) -> p h x", h=nh)[:, :, 0:HD]
                        nc.scalar.activation(
                            dst, src, mybir.ActivationFunctionType.Copy)

            # ---- main attention loop ----
            with tc.tile_pool(name="sps", bufs=2, space="PSUM") as spsp, \
                 tc.tile_pool(name="ops", bufs=1, space="PSUM") as opsp, \
                 tc.tile_pool(name="p2p", bufs=2, space="PSUM") as p2p:
                for i2 in range(2):
                    for hp in range(NHP):
                        hA, hB = 2 * hp, 2 * hp + 1
                        kt, qt = kT[hp], qT[hp]
                        oAB = [opsp.tile([P, 512], F32, tag=f"o{x}", name=f"o{x}_{hp}_{i2}")
                               for x in "AB"]
                        for j in range(JC):
                            S2 = spsp.tile([P, 1024], F32, tag="S2")
                            nc.tensor.matmul(
                                S2[:, 0:512],
                                kt[0:HD, j * P:(j + 1) * P],
                                qt[0:HD, i2 * 512:(i2 + 1) * 512],
                                start=True, stop=True, tile_position=(0, 0))
                            nc.tensor.matmul(
                                S2[:, 512:1024],
                                kt[HD:P, j * P:(j + 1) * P],
                                qt[HD:P, i2 * 512:(i2 + 1) * 512],
                                start=True, stop=True, tile_position=(64, 0))
                            EX = work.tile([P, 1024], BF16, tag="EX")
                            nc.scalar.activation(
                                EX[:], S2[:],
                                mybir.ActivationFunctionType.Exp, scale=0.125)
                            XM = work.tile([P, 1024], BF16, tag="XM")
                            base = i2 * 4096 + j * 512
                            m_ap = mT[:, base:base + 512] \
                                .unsqueeze(1).broadcast_to((P, 2, 512))
                            nc.vector.tensor_tensor(
                                out=XM[:].rearrange("p (h x) -> p h x", h=2),
                                in0=EX[:].rearrange("p (h x) -> p h x", h=2),
                                in1=m_ap, op=mybir.AluOpType.mult)
                            first, last = (j == 0), (j == JC - 1)
                            nc.tensor.matmul(
                                oAB[0][0:VW, :],
                                vp[j][:, hA * VW:(hA + 1) * VW],
                                XM[:, 0:512], start=first, stop=last)
                            nc.tensor.matmul(
                                oAB[1][0:VW, :],
                                vp[j][:, hB * VW:(hB + 1) * VW],
                                XM[:, 512:1024], start=first, stop=last)
                        # output: transpose + normalize per head
                        for h, o in ((hA, oAB[0]), (hB, oAB[1])):
                            oraw = work.tile([P, 512], BF16, tag="oraw")
                            nc.vector.tensor_copy(out=oraw[0:VW, :], in_=o[0:VW, :])
                            P2 = p2p.tile([P, 4 * SW], BF16, tag="P2")
                            for s in range(4):
                                nc.tensor.transpose(
                                    P2[:, s * SW:s * SW + VW],
                                    oraw[0:VW, s * P:(s + 1) * P],
                                    ident[0:VW, 0:VW])
                            rec = work.tile([P, 4], BF16, tag="rec")
                            den_ap = P2[:].rearrange("p (s x) -> p s x", s=4)[:, :, HD:HD + 1]
                            with nc.allow_low_precision(reason="bf16 softmax denom is ample"):
                                nc.vector.tensor_scalar_max(rec[:].unsqueeze(2), den_ap, 1e-9)
                                nc.vector.reciprocal(rec[:], rec[:])
                            src = P2[:].rearrange("p (s x) -> p s x", s=4)[:, :, 0:HD]
                            r_b = rec[:].unsqueeze(2).broadcast_to((P, 4, HD))
                            dst = outt[:, i2 * 4 * H:(i2 + 1) * 4 * H] \
                                .rearrange("p (s x) -> p s x", s=4)[:, :, h * HD:(h + 1) * HD]
                            nc.vector.tensor_tensor(
                                out=dst, in0=src, in1=r_b, op=mybir.AluOpType.mult)
                    # this i-half is complete for all heads: stream it out
                    for s in range(4):
                        ic = i2 * 4 + s
                        nc.default_dma_engine.dma_start(
                            out=out_d[ic * P:(ic + 1) * P, :],
                            in_=outt[:, ic * H:(ic + 1) * H])

    nc.compile()
    return nc


def _prep_in_maps(node_states, edge_indices, Wq, Wk, Wv):
    eb, ei, ej = (np.asarray(edge_indices[r]) for r in range(3))
    idx = (eb.astype(np.int64) * N + ej) * N + ei
    CT = np.bincount(idx, minlength=B * N * N).astype(np.float32).reshape(B, N, N)
    # device mask layout: [p, i2*4096 + jc*512 + io]
    CTd = CT.reshape(B, JC, P, 2, 512).transpose(0, 2, 3, 1, 4).reshape(B, P, JC * N)

    bf = ml_dtypes.bfloat16
    ident = np.eye(P, dtype=bf)
    wq = np.ascontiguousarray(Wq).astype(bf)
    wk = np.ascontiguousarray(Wk).astype(bf)
    wv = np.ascontiguousarray(Wv).astype(bf)

    in_maps = []
    for b in range(B):
        in_maps.append({
            "xT": np.ascontiguousarray(np.asarray(node_states[b]).T).astype(bf),
            "wq": wq, "wk": wk, "wv": wv,
            "maskT": np.ascontiguousarray(CTd[b]).astype(bf),
            "ident": ident,
        })
    return in_maps


def kernel(node_states, edge_indices, Wq, Wk, Wv):
    if "nc" not in _CACHE:
        _CACHE["nc"] = _build_nc()
    nc = _CACHE["nc"]
    in_maps = _prep_in_maps(node_states, edge_indices, Wq, Wk, Wv)
    res = run_bass_kernel_spmd(nc, in_maps, list(range(B)))
    out = np.stack([np.asarray(res.results[b]["out"]) for b in range(B)], axis=0)
    return out.astype(np.float32)


def run_traced(inputs, **kw):
    if "nc" not in _CACHE:
        _CACHE["nc"] = _build_nc()
    nc = _CACHE["nc"]
    in_maps = _prep_in_maps(**inputs)
    return run_bass_kernel_spmd(nc, in_maps, list(range(B)), trace=True, **kw)


# revision 16
# speedup vs baseline: 1.4900x; 1.0340x over previous
"""GAT-style edge-softmax self-attention, dense-mask formulation, 8-core SPMD.

Math: per batch b (one NeuronCore per batch),
  Q/K/V = X @ Wq/k/v ; per head h: S = Q_h K_h^T / 8
  ex = C * exp(S)           (C[i,j] = multiplicity of edge (i<-j); softmax is
                             shift-invariant and |S| <~ 7, so no row-max needed)
  out_i = (ex @ V)_i / max(sum_j ex_ij, 1e-9)

v6 design notes (per core):
  - scores: head PAIRS via PE row tiling (K=64, tile_position (0,0)/(64,0))
    into a 2-bank PSUM group, double-buffered so scores of chunk j+1 overlap
    exp/mult/AV of chunk j.
  - exp: one ACT instruction per 2-bank group (free dim 1024), reading PSUM.
  - mask multiply: all-bf16 tensor_tensor (DVE 2x), mask block broadcast
    across the 2 heads via a stride-0 AP dim.
  - AV: V stationary (64 cols + ones column -> softmax denominator), exm
    streams n=512.  Output [feature, node] in PSUM.
  - output: DVE copy to bf16 SBUF, 4x PE transpose back to [node, feature]
    (66-col blocks keep bf16 PSUM writes 4B-aligned), reciprocal-normalize
    with a stride-0 broadcast TT into a bf16 staging buffer; each i-half is
    DMA'd out as soon as all heads finished it.  Host upcasts to float32.
"""

import numpy as np
import ml_dtypes

import concourse.bass as bass
import concourse.bacc as bacc
import concourse.mybir as mybir
import concourse.tile as tile
from concourse.bass_utils import run_bass_kernel_spmd

B, N, H = 8, 1024, 768
NH, HD = 12, 64
P = 128
KC = H // P   # 6 contraction chunks for projections
JC = N // P   # 8 node chunks
NHP = NH // 2  # head pairs
VW = HD + 1    # per-head stride in vp (V cols + ones col)
SW = HD + 2    # per-s block stride in the transposed PSUM tile (4B aligned)
F32 = mybir.dt.float32
BF16 = mybir.dt.bfloat16

_CACHE = {}


def _build_nc():
    nc = bacc.Bacc("TRN2", target_bir_lowering=False, debug=True)

    xT_d = nc.dram_tensor("xT", [H, N], BF16, kind="ExternalInput")
    wq_d = nc.dram_tensor("wq", [H, H], BF16, kind="ExternalInput")
    wk_d = nc.dram_tensor("wk", [H, H], BF16, kind="ExternalInput")
    wv_d = nc.dram_tensor("wv", [H, H], BF16, kind="ExternalInput")
    # mask, device layout: [p, i2*4096 + jc*512 + io] (j = jc*128+p, i = i2*512+io)
    mT_d = nc.dram_tensor("maskT", [P, JC * N], BF16, kind="ExternalInput")
    id_d = nc.dram_tensor("ident", [P, P], BF16, kind="ExternalInput")
    out_d = nc.dram_tensor("out", [N, H], BF16, kind="ExternalOutput")

    with tile.TileContext(nc) as tc:
        with tc.tile_pool(name="res", bufs=1) as res, \
             tc.tile_pool(name="work", bufs=3) as work, \
             tc.tile_pool(name="pps", bufs=2, space="PSUM") as pps, \
             tc.tile_pool(name="sps", bufs=2, space="PSUM") as spsp, \
             tc.tile_pool(name="ops", bufs=1, space="PSUM") as opsp:

            # ---- resident loads ----
            xT = [res.tile([P, N], BF16, tag=f"xT{k}", name=f"xT{k}") for k in range(KC)]
            wq = [res.tile([P, H], BF16, tag=f"wq{k}", name=f"wq{k}") for k in range(KC)]
            wk = [res.tile([P, H], BF16, tag=f"wk{k}", name=f"wk{k}") for k in range(KC)]
            wv = [res.tile([P, H], BF16, tag=f"wv{k}", name=f"wv{k}") for k in range(KC)]
            mT = res.tile([P, JC * N], BF16, tag="mT", name="mT")
            ident = res.tile([P, P], BF16, tag="ident", name="ident")
            nc.default_dma_engine.dma_start(out=ident[:], in_=id_d[:, :])
            for k in range(KC):
                nc.default_dma_engine.dma_start(out=xT[k][:], in_=xT_d[k * P:(k + 1) * P, :])
                nc.default_dma_engine.dma_start(out=wq[k][:], in_=wq_d[k * P:(k + 1) * P, :])
                nc.default_dma_engine.dma_start(out=wk[k][:], in_=wk_d[k * P:(k + 1) * P, :])
                nc.default_dma_engine.dma_start(out=wv[k][:], in_=wv_d[k * P:(k + 1) * P, :])
            for j in range(JC):
                nc.default_dma_engine.dma_start(
                    out=mT[:, j * N:(j + 1) * N], in_=mT_d[:, j * N:(j + 1) * N])

            # computed residents
            qT = [res.tile([P, N], BF16, tag=f"qT{k}", name=f"qT{k}") for k in range(KC)]
            kT = [res.tile([P, N], BF16, tag=f"kT{k}", name=f"kT{k}") for k in range(KC)]
            vp = [res.tile([P, NH * VW], BF16, tag=f"vp{j}", name=f"vp{j}") for j in range(JC)]
            # final output, [p, ic*768 + h*64 + f], bf16
            outt = res.tile([P, JC * H], BF16, tag="outt", name="outt")

            # ---- projections: V first, then Q/K chunk 0; remaining Q/K
            # chunks are emitted inside the first attention pass so the PE's
            # in-order stream interleaves them with attention matmuls.
            for j in range(JC):
                nc.gpsimd.memset(vp[j][:], 1.0)
                for nn, (c0, cw, nh) in enumerate(((0, 512, 8), (512, 256, 4))):
                    ps = pps.tile([P, 512], F32, tag="proj")
                    for k in range(KC):
                        nc.tensor.matmul(
                            ps[:, :cw],
                            xT[k][:, j * P:(j + 1) * P],
                            wv[k][:, c0:c0 + cw],
                            start=(k == 0), stop=(k == KC - 1),
                        )
                    h0 = c0 // HD
                    srcv = ps[:, 0:cw].rearrange("p (h x) -> p h x", h=nh)
                    dstv = vp[j][:, h0 * VW:(h0 + nh) * VW] \
                        .rearrange("p (h x) -> p h x", h=nh)[:, :, 0:HD]
                    nc.scalar.activation(
                        dstv, srcv, mybir.ActivationFunctionType.Copy)

            def qk_chunk(mo):
                for w_sb, dst in ((wq, qT), (wk, kT)):
                    for nn in range(2):
                        ps = pps.tile([P, 512], F32, tag="proj")
                        for k in range(KC):
                            nc.tensor.matmul(
                                ps[:],
                                w_sb[k][:, mo * P:(mo + 1) * P],
                                xT[k][:, nn * 512:(nn + 1) * 512],
                                start=(k == 0), stop=(k == KC - 1),
                            )
                        nc.scalar.activation(
                            dst[mo][:, nn * 512:(nn + 1) * 512], ps[:],
                            mybir.ActivationFunctionType.Copy)

            qk_chunk(0)

            # ---- main attention loop ----
            if True:
                for i2 in range(2):
                    for hp in range(NHP):
                        hA, hB = 2 * hp, 2 * hp + 1
                        kt, qt = kT[hp], qT[hp]
                        oAB = [opsp.tile([P, 512], F32, tag=f"o{x}", name=f"o{x}_{hp}_{i2}")
                               for x in "AB"]
                        for j in range(JC):
                            S2 = spsp.tile([P, 1024], F32, tag="S2")
                            nc.tensor.matmul(
                                S2[:, 0:512],
                                kt[0:HD, j * P:(j + 1) * P],
                                qt[0:HD, i2 * 512:(i2 + 1) * 512],
                                start=True, stop=True, tile_position=(0, 0))
                            nc.tensor.matmul(
                                S2[:, 512:1024],
                                kt[HD:P, j * P:(j + 1) * P],
                                qt[HD:P, i2 * 512:(i2 + 1) * 512],
                                start=True, stop=True, tile_position=(64, 0))
                            EX = work.tile([P, 1024], BF16, tag="EX")
                            nc.scalar.activation(
                                EX[:], S2[:],
                                mybir.ActivationFunctionType.Exp, scale=0.125)
                            XM = work.tile([P, 1024], BF16, tag="XM")
                            base = i2 * 4096 + j * 512
                            m_ap = mT[:, base:base + 512] \
                                .unsqueeze(1).broadcast_to((P, 2, 512))
                            nc.vector.tensor_tensor(
                                out=XM[:].rearrange("p (h x) -> p h x", h=2),
                                in0=EX[:].rearrange("p (h x) -> p h x", h=2),
                                in1=m_ap, op=mybir.AluOpType.mult)
                            first, last = (j == 0), (j == JC - 1)
                            nc.tensor.matmul(
                                oAB[0][0:VW, :],
                                vp[j][:, hA * VW:(hA + 1) * VW],
                                XM[:, 0:512], start=first, stop=last)
                            nc.tensor.matmul(
                                oAB[1][0:VW, :],
                                vp[j][:, hB * VW:(hB + 1) * VW],
                                XM[:, 512:1024], start=first, stop=last)
                        # output: transpose + normalize per head
                        for h, (o, otag) in ((hA, (oAB[0], "oA")), (hB, (oAB[1], "oB"))):
                            oraw = work.tile([P, 512], BF16, tag="oraw")
                            nc.vector.tensor_copy(out=oraw[0:VW, :], in_=o[0:VW, :])
                            P2 = opsp.tile([P, 4 * SW], BF16, tag=otag,
                                           name=f"P2_{h}_{i2}")
                            for s in range(4):
                                nc.tensor.transpose(
                                    P2[:, s * SW:s * SW + VW],
                                    oraw[0:VW, s * P:(s + 1) * P],
                                    ident[0:VW, 0:VW])
                            rec = work.tile([P, 4], BF16, tag="rec")
                            den_ap = P2[:].rearrange("p (s x) -> p s x", s=4)[:, :, HD:HD + 1]
                            with nc.allow_low_precision(reason="bf16 softmax denom is ample"):
                                nc.vector.tensor_scalar_max(rec[:].unsqueeze(2), den_ap, 1e-9)
                                nc.vector.reciprocal(rec[:], rec[:])
                            src = P2[:].rearrange("p (s x) -> p s x", s=4)[:, :, 0:HD]
                            r_b = rec[:].unsqueeze(2).broadcast_to((P, 4, HD))
                            dst = outt[:, i2 * 4 * H:(i2 + 1) * 4 * H] \
                                .rearrange("p (s x) -> p s x", s=4)[:, :, h * HD:(h + 1) * HD]
                            nc.vector.tensor_tensor(
                                out=dst, in0=src, in1=r_b, op=mybir.AluOpType.mult)
                        if i2 == 0 and hp + 1 < KC:
                            qk_chunk(hp + 1)  # interleave remaining projections
                    # this i-half is complete for all heads: stream it out
                    for s in range(4):
                        ic = i2 * 4 + s
                        nc.default_dma_engine.dma_start(
                            out=out_d[ic * P:(ic + 1) * P, :],
                            in_=outt[:, ic * H:(ic + 1) * H])

    nc.compile()
    return nc


def _prep_in_maps(node_states, edge_indices, Wq, Wk, Wv):
    eb, ei, ej = (np.asarray(edge_indices[r]) for r in range(3))
    idx = (eb.astype(np.int64) * N + ej) * N + ei
    CT = np.bincount(idx, minlength=B * N * N).astype(np.float32).reshape(B, N, N)
    # device mask layout: [p, i2*4096 + jc*512 + io]
    CTd = CT.reshape(B, JC, P, 2, 512).transpose(0, 2, 3, 1, 4).reshape(B, P, JC * N)

    bf = ml_dtypes.bfloat16
    ident = np.eye(P, dtype=bf)
    wq = np.ascontiguousarray(Wq).astype(bf)
    wk = np.ascontiguousarray(Wk).astype(bf)
    wv = np.ascontiguousarray(Wv).astype(bf)

    in_maps = []
    for b in range(B):
        in_maps.append({
            "xT": np.ascontiguousarray(np.asarray(node_states[b]).T).astype(bf),
            "wq": wq, "wk": wk, "wv": wv,
            "maskT": np.ascontiguousarray(CTd[b]).astype(bf),
            "ident": ident,
        })
    return in_maps


def kernel(node_states, edge_indices, Wq, Wk, Wv):
    if "nc" not in _CACHE:
        _CACHE["nc"] = _build_nc()
    nc = _CACHE["nc"]
    in_maps = _prep_in_maps(node_states, edge_indices, Wq, Wk, Wv)
    res = run_bass_kernel_spmd(nc, in_maps, list(range(B)))
    out = np.stack([np.asarray(res.results[b]["out"]) for b in range(B)], axis=0)
    return out.astype(np.float32)


def run_traced(inputs, **kw):
    if "nc" not in _CACHE:
        _CACHE["nc"] = _build_nc()
    nc = _CACHE["nc"]
    in_maps = _prep_in_maps(**inputs)
    return run_bass_kernel_spmd(nc, in_maps, list(range(B)), trace=True, **kw)


# revision 17
# speedup vs baseline: 1.5755x; 1.0573x over previous
"""GAT-style edge-softmax self-attention, dense-mask formulation, 8-core SPMD.

Math: per batch b (one NeuronCore per batch),
  Q/K/V = X @ Wq/k/v ; per head h: S = Q_h K_h^T / 8
  ex = C * exp(S)           (C[i,j] = multiplicity of edge (i<-j); softmax is
                             shift-invariant and |S| <~ 7, so no row-max needed)
  out_i = (ex @ V)_i / max(sum_j ex_ij, 1e-9)

v6 design notes (per core):
  - scores: head PAIRS via PE row tiling (K=64, tile_position (0,0)/(64,0))
    into a 2-bank PSUM group, double-buffered so scores of chunk j+1 overlap
    exp/mult/AV of chunk j.
  - exp: one ACT instruction per 2-bank group (free dim 1024), reading PSUM.
  - mask multiply: all-bf16 tensor_tensor (DVE 2x), mask block broadcast
    across the 2 heads via a stride-0 AP dim.
  - AV: V stationary (64 cols + ones column -> softmax denominator), exm
    streams n=512.  Output [feature, node] in PSUM.
  - output: DVE copy to bf16 SBUF, 4x PE transpose back to [node, feature]
    (66-col blocks keep bf16 PSUM writes 4B-aligned), reciprocal-normalize
    with a stride-0 broadcast TT into a bf16 staging buffer; each i-half is
    DMA'd out as soon as all heads finished it.  Host upcasts to float32.
"""

import numpy as np
import ml_dtypes

import concourse.bass as bass
import concourse.bacc as bacc
import concourse.mybir as mybir
import concourse.tile as tile
from concourse.bass_utils import run_bass_kernel_spmd

B, N, H = 8, 1024, 768
NH, HD = 12, 64
P = 128
KC = H // P   # 6 contraction chunks for projections
JC = N // P   # 8 node chunks
NHP = NH // 2  # head pairs
VW = HD + 1    # per-head stride in vp (V cols + ones col)
SW = HD + 2    # per-s block stride in the transposed PSUM tile (4B aligned)
F32 = mybir.dt.float32
BF16 = mybir.dt.bfloat16

_CACHE = {}


def _build_nc():
    nc = bacc.Bacc("TRN2", target_bir_lowering=False, debug=True)

    xT_d = nc.dram_tensor("xT", [H, N], BF16, kind="ExternalInput")
    wq_d = nc.dram_tensor("wq", [H, H], BF16, kind="ExternalInput")
    wk_d = nc.dram_tensor("wk", [H, H], BF16, kind="ExternalInput")
    wv_d = nc.dram_tensor("wv", [H, H], BF16, kind="ExternalInput")
    # mask, device layout: [p, i2*4096 + jc*512 + io] (j = jc*128+p, i = i2*512+io)
    mT_d = nc.dram_tensor("maskT", [P, JC * N], BF16, kind="ExternalInput")
    id_d = nc.dram_tensor("ident", [P, P], BF16, kind="ExternalInput")
    out_d = nc.dram_tensor("out", [N, H], BF16, kind="ExternalOutput")

    with tile.TileContext(nc) as tc:
        with tc.tile_pool(name="res", bufs=1) as res, \
             tc.tile_pool(name="work", bufs=3) as work, \
             tc.tile_pool(name="pps", bufs=2, space="PSUM") as pps, \
             tc.tile_pool(name="sps", bufs=2, space="PSUM") as spsp, \
             tc.tile_pool(name="ops", bufs=1, space="PSUM") as opsp:

            # ---- resident loads ----
            xT = [res.tile([P, N], BF16, tag=f"xT{k}", name=f"xT{k}") for k in range(KC)]
            wq = [res.tile([P, H], BF16, tag=f"wq{k}", name=f"wq{k}") for k in range(KC)]
            wk = [res.tile([P, H], BF16, tag=f"wk{k}", name=f"wk{k}") for k in range(KC)]
            wv = [res.tile([P, H], BF16, tag=f"wv{k}", name=f"wv{k}") for k in range(KC)]
            mT = res.tile([P, JC * N], BF16, tag="mT", name="mT")
            ident = res.tile([P, P], BF16, tag="ident", name="ident")
            for k in range(KC):
                nc.default_dma_engine.dma_start(out=xT[k][:], in_=xT_d[k * P:(k + 1) * P, :])
                nc.default_dma_engine.dma_start(out=wv[k][:], in_=wv_d[k * P:(k + 1) * P, :])
            for k in range(KC):
                nc.default_dma_engine.dma_start(out=wq[k][:], in_=wq_d[k * P:(k + 1) * P, :])
                nc.default_dma_engine.dma_start(out=wk[k][:], in_=wk_d[k * P:(k + 1) * P, :])
            nc.default_dma_engine.dma_start(out=ident[:], in_=id_d[:, :])
            for j in range(JC):
                nc.default_dma_engine.dma_start(
                    out=mT[:, j * N:(j + 1) * N], in_=mT_d[:, j * N:(j + 1) * N])

            # computed residents
            qT = [res.tile([P, N], BF16, tag=f"qT{k}", name=f"qT{k}") for k in range(KC)]
            kT = [res.tile([P, N], BF16, tag=f"kT{k}", name=f"kT{k}") for k in range(KC)]
            vp = [res.tile([P, NH * VW], BF16, tag=f"vp{j}", name=f"vp{j}") for j in range(JC)]
            # final output, [p, ic*768 + h*64 + f], bf16
            outt = res.tile([P, JC * H], BF16, tag="outt", name="outt")

            # ---- projections, emitted lazily inside the attention pass so
            # the PE's in-order stream interleaves them with attention matmuls.
            def v_chunk(j):
                nc.gpsimd.memset(vp[j][:], 1.0)
                for nn, (c0, cw, nh) in enumerate(((0, 512, 8), (512, 256, 4))):
                    ps = pps.tile([P, 512], F32, tag="proj")
                    for k in range(KC):
                        nc.tensor.matmul(
                            ps[:, :cw],
                            xT[k][:, j * P:(j + 1) * P],
                            wv[k][:, c0:c0 + cw],
                            start=(k == 0), stop=(k == KC - 1),
                        )
                    h0 = c0 // HD
                    srcv = ps[:, 0:cw].rearrange("p (h x) -> p h x", h=nh)
                    dstv = vp[j][:, h0 * VW:(h0 + nh) * VW] \
                        .rearrange("p (h x) -> p h x", h=nh)[:, :, 0:HD]
                    nc.scalar.activation(
                        dstv, srcv, mybir.ActivationFunctionType.Copy)

            def qk_chunk(mo):
                for w_sb, dst in ((wq, qT), (wk, kT)):
                    for nn in range(2):
                        ps = pps.tile([P, 512], F32, tag="proj")
                        for k in range(KC):
                            nc.tensor.matmul(
                                ps[:],
                                w_sb[k][:, mo * P:(mo + 1) * P],
                                xT[k][:, nn * 512:(nn + 1) * 512],
                                start=(k == 0), stop=(k == KC - 1),
                            )
                        nc.scalar.activation(
                            dst[mo][:, nn * 512:(nn + 1) * 512], ps[:],
                            mybir.ActivationFunctionType.Copy)

            qk_chunk(0)

            # ---- main attention loop ----
            if True:
                for hp in range(NHP):
                    for i2 in range(2):
                        hA, hB = 2 * hp, 2 * hp + 1
                        kt, qt = kT[hp], qT[hp]
                        oAB = [opsp.tile([P, 512], F32, tag=f"o{x}", name=f"o{x}_{hp}_{i2}")
                               for x in "AB"]
                        for j in range(JC):
                            if hp == 0 and i2 == 0:
                                v_chunk(j)  # just-in-time V projection
                            S2 = spsp.tile([P, 1024], F32, tag="S2")
                            nc.tensor.matmul(
                                S2[:, 0:512],
                                kt[0:HD, j * P:(j + 1) * P],
                                qt[0:HD, i2 * 512:(i2 + 1) * 512],
                                start=True, stop=True, tile_position=(0, 0))
                            nc.tensor.matmul(
                                S2[:, 512:1024],
                                kt[HD:P, j * P:(j + 1) * P],
                                qt[HD:P, i2 * 512:(i2 + 1) * 512],
                                start=True, stop=True, tile_position=(64, 0))
                            EX = work.tile([P, 1024], BF16, tag="EX")
                            nc.scalar.activation(
                                EX[:], S2[:],
                                mybir.ActivationFunctionType.Exp, scale=0.125)
                            XM = work.tile([P, 1024], BF16, tag="XM")
                            base = i2 * 4096 + j * 512
                            m_ap = mT[:, base:base + 512] \
                                .unsqueeze(1).broadcast_to((P, 2, 512))
                            nc.vector.tensor_tensor(
                                out=XM[:].rearrange("p (h x) -> p h x", h=2),
                                in0=EX[:].rearrange("p (h x) -> p h x", h=2),
                                in1=m_ap, op=mybir.AluOpType.mult)
                            first, last = (j == 0), (j == JC - 1)
                            nc.tensor.matmul(
                                oAB[0][0:VW, :],
                                vp[j][:, hA * VW:(hA + 1) * VW],
                                XM[:, 0:512], start=first, stop=last)
                            nc.tensor.matmul(
                                oAB[1][0:VW, :],
                                vp[j][:, hB * VW:(hB + 1) * VW],
                                XM[:, 512:1024], start=first, stop=last)
                        # output: transpose + normalize per head
                        for h, (o, otag) in ((hA, (oAB[0], "oA")), (hB, (oAB[1], "oB"))):
                            oraw = work.tile([P, 512], BF16, tag="oraw")
                            nc.vector.tensor_copy(out=oraw[0:VW, :], in_=o[0:VW, :])
                            P2 = opsp.tile([P, 4 * SW], BF16, tag=otag,
                                           name=f"P2_{h}_{i2}")
                            for s in range(4):
                                nc.tensor.transpose(
                                    P2[:, s * SW:s * SW + VW],
                                    oraw[0:VW, s * P:(s + 1) * P],
                                    ident[0:VW, 0:VW])
                            rec = work.tile([P, 4], BF16, tag="rec")
                            den_ap = P2[:].rearrange("p (s x) -> p s x", s=4)[:, :, HD:HD + 1]
                            with nc.allow_low_precision(reason="bf16 softmax denom is ample"):
                                nc.vector.tensor_scalar_max(rec[:].unsqueeze(2), den_ap, 1e-9)
                                nc.vector.reciprocal(rec[:], rec[:])
                            src = P2[:].rearrange("p (s x) -> p s x", s=4)[:, :, 0:HD]
                            r_b = rec[:].unsqueeze(2).broadcast_to((P, 4, HD))
                            dst = outt[:, i2 * 4 * H:(i2 + 1) * 4 * H] \
                                .rearrange("p (s x) -> p s x", s=4)[:, :, h * HD:(h + 1) * HD]
                            nc.vector.tensor_tensor(
                                out=dst, in0=src, in1=r_b, op=mybir.AluOpType.mult)
                        if i2 == 1 and hp + 1 < KC:
                            qk_chunk(hp + 1)  # interleave remaining projections
                for ic in range(JC):
                    nc.default_dma_engine.dma_start(
                        out=out_d[ic * P:(ic + 1) * P, :],
                        in_=outt[:, ic * H:(ic + 1) * H])

    nc.compile()
    return nc


def _prep_in_maps(node_states, edge_indices, Wq, Wk, Wv):
    eb, ei, ej = (np.asarray(edge_indices[r]) for r in range(3))
    idx = (eb.astype(np.int64) * N + ej) * N + ei
    CT = np.bincount(idx, minlength=B * N * N).astype(np.float32).reshape(B, N, N)
    # device mask layout: [p, i2*4096 + jc*512 + io]
    CTd = CT.reshape(B, JC, P, 2, 512).transpose(0, 2, 3, 1, 4).reshape(B, P, JC * N)

    bf = ml_dtypes.bfloat16
    ident = np.eye(P, dtype=bf)
    wq = np.ascontiguousarray(Wq).astype(bf)
    wk = np.ascontiguousarray(Wk).astype(bf)
    wv = np.ascontiguousarray(Wv).astype(bf)

    in_maps = []
    for b in range(B):
        in_maps.append({
            "xT": np.ascontiguousarray(np.asarray(node_states[b]).T).astype(bf),
            "wq": wq, "wk": wk, "wv": wv,
            "maskT": np.ascontiguousarray(CTd[b]).astype(bf),
            "ident": ident,
        })
    return in_maps


def kernel(node_states, edge_indices, Wq, Wk, Wv):
    if "nc" not in _CACHE:
        _CACHE["nc"] = _build_nc()
    nc = _CACHE["nc"]
    in_maps = _prep_in_maps(node_states, edge_indices, Wq, Wk, Wv)
    res = run_bass_kernel_spmd(nc, in_maps, list(range(B)))
    out = np.stack([np.asarray(res.results[b]["out"]) for b in range(B)], axis=0)
    return out.astype(np.float32)


def run_traced(inputs, **kw):
    if "nc" not in _CACHE:
        _CACHE["nc"] = _build_nc()
    nc = _CACHE["nc"]
    in_maps = _prep_in_maps(**inputs)
    return run_bass_kernel_spmd(nc, in_maps, list(range(B)), trace=True, **kw)
